# revision 20
# baseline (speedup 1.0000x reference)
"""Trainium2 Bass kernel for nn_AttShare: dual-stream 1x1-conv attention.

Full-input contract: kernel(**inputs) takes the complete tensors from
setup_inputs() and returns (out1, out2) exactly like the reference.

Sharding (8 cores): 4 independent (batch, stream) attention units x 2-way
query-row split.  Each core gets the full x=[256,4096] of its unit, HOST-
ROTATED so its 2048 query columns come first; it produces
out = gamma * (V @ softmax(Q K^T)^T)[:, 0:2048] + x[:, 0:2048].
(Attention contracts over all keys, so the key/value column order is
irrelevant; the host scatters the output back to the right columns.)

Key simplification: the reference adds a per-row bias (q . g) to the logits
before a row-softmax.  softmax is shift-invariant per row, so the entire
global-gating branch (pooled means -> MLP -> sigmoid -> bias) cancels and is
not computed.  The k-projection bias also shifts logits uniformly per row
and cancels; the q bias does not and is applied.  The v bias adds
gamma*vb[c] (softmax rows sum to 1); it is folded into the V^T tiles.

Precision: projections and QK logits run in float32r (full fp32 weights,
~19-bit moving operand) -- logit errors get exponentiated, so this path
stays wide.  The PV path (V^T tiles and exp tiles) runs in bfloat16:
weight loads take 1 pass instead of fp32's 2, cutting the ldweights
exposure between back-to-back PV matmuls.  Measured accuracy impact
~1.4e-3 relative (tolerance 2e-2).

On-core dataflow (per core):
  proj:  qq = Wq_dup @ x[:, :2048] (+qb)  [128, 2048] f32r  (q/k duplicated
         kk = Wk_dup @ x  (+kb)           [128, 4096] f32r   on both halves
         vt = gamma*(x^T @ Wv^T + vb)     [128 j, 32, 256] bf16  for packing)
  attn (2 phases of 1024 query columns, j streamed in row-packed pairs,
        software-pipelined one pair ahead):
         S^T tile = kk_j^T @ qq  (K=64, rows 0-63 / 64-127 concurrently)
         E = exp(S^T)  (ScalarE, PSUM -> bf16 SBUF; no max-shift needed:
                        |S|<~60 and the denominator normalizes later)
         ZA += E_A (Pool)   ZB += E_B (Vector)   [split across engines]
         out_psum[c,i] += vt_j^T @ E  (bf16 matmuls, PSUM-resident)
  finalize per 512-col slice (pipelined against the next phase / the last
  PV matmuls): Z colsum+broadcast via all-ones lhsT matmuls (4 terms: ZA,
  ZB and the last pair's exp tiles summed directly by the PE), reciprocal
  (Vector), out = out_psum * recip (Vector) + x (Pool), DMA out (SP/ACT
  queues alternating).
  PSUM budget 8 banks: 4 output accumulators + 2x2-bank S^T tiles.

Head: input DMA issues are split across the SP and Activation hardware
queues (plus Pool's software queue for the scalars) so descriptor
generation (~0.7us each) does not serialize the x stream.
"""

import os
import sys

import numpy as np

for _p in ("/opt/trn_rl_repo", os.path.expanduser("~/.axon_site/_ro/trn_rl_repo")):
    if os.path.isdir(_p) and _p not in sys.path:
        sys.path.insert(0, _p)

import concourse.bass as bass  # noqa: E402
import concourse.bacc as bacc  # noqa: E402
import concourse.mybir as mybir  # noqa: E402
import concourse.tile as tile  # noqa: E402

P = 128
C = 256         # channels
CQ = 64         # q/k channels
N = 4096        # H*W
NI = 2048       # query rows per core
PH = 512        # query columns processed per phase
B, H, W = 2, 64, 64
F32 = mybir.dt.float32
BF16 = mybir.dt.bfloat16
MM_DT = mybir.dt.float32r


def _f(ap):
    """View a float32r AP as plain fp32 (for non-matmul engine access)."""
    return ap.bitcast(F32)


def _r(ap):
    """View an fp32 AP as float32r (for matmul operands)."""
    return ap.bitcast(MM_DT)


def _emit(tc, aps):
    nc = tc.nc
    import contextlib

    x_d, wq_d, wk_d, wv_d, qb_d, kb_d, vb_d, gamma_d, out_d = aps
    EXP = mybir.ActivationFunctionType.Exp
    IDENT = mybir.ActivationFunctionType.Identity

    with contextlib.ExitStack() as ctx:
        singles = ctx.enter_context(tc.tile_pool(name="singles", bufs=1))
        pp = ctx.enter_context(tc.tile_pool(name="pp", bufs=4, space="PSUM"))
        p_s = ctx.enter_context(tc.tile_pool(name="p_s", bufs=2, space="PSUM"))
        etp = ctx.enter_context(tc.tile_pool(name="etp", bufs=8))
        zp = ctx.enter_context(tc.tile_pool(name="zp", bufs=4))
        outp = ctx.enter_context(tc.tile_pool(name="outp", bufs=4))

        # ---- loads --------------------------------------------------------------
        x_sb = singles.tile([P, 2, N], MM_DT)
        wq_sb = singles.tile([P, 2, P], MM_DT)
        wk_sb = singles.tile([P, 2, P], MM_DT)
        wv_sb = singles.tile([P, 2, C], MM_DT)
        x_r = x_d[:].rearrange("(o p) n -> p o n", p=P)

        gamma_sb = singles.tile([1, 1], F32)
        kb_sb = singles.tile([P, 1], F32)
        qb_sb = singles.tile([P, 1], F32)
        vb_sb = singles.tile([1, C], F32)

        # gamma/vb lead the SP queue (tiny transfers, unblock the HAM-warming
        # broadcast matmuls); qb/kb ride the Pool software queue.  The x
        # stream is split by cin-half across the two hardware DMA rings
        # (SP and Activation, ~150GB/s each) so each chunk's halves arrive
        # together at the aggregate ~300GB/s.
        nc.sync.dma_start(out=gamma_sb, in_=gamma_d[:])
        nc.sync.dma_start(out=vb_sb, in_=vb_d[:])
        nc.gpsimd.dma_start(out=qb_sb, in_=qb_d[:])
        nc.gpsimd.dma_start(out=kb_sb, in_=kb_d[:])

        def ld_x_half(queue, c, o):
            queue.dma_start(out=x_sb[:, o:o + 1, bass.ts(c, N // 8)],
                            in_=x_r[:, o:o + 1, bass.ts(c, N // 8)])

        def ld_x_2(queue, c, o):
            queue.dma_start(out=x_sb[:, o:o + 1, bass.ds(c * 512, 1024)],
                            in_=x_r[:, o:o + 1, bass.ds(c * 512, 1024)])

        # the Activation ring's issue instructions occupy ScalarE, so its
        # transfers are fused into fewer, larger descriptors
        nc.scalar.dma_start(out=wq_sb, in_=wq_d[:].rearrange("(o p) m -> p o m", p=P))
        ld_x_half(nc.sync, 0, 0)
        ld_x_half(nc.scalar, 0, 1)
        ld_x_half(nc.sync, 1, 0)
        nc.scalar.dma_start(out=wk_sb, in_=wk_d[:].rearrange("(o p) m -> p o m", p=P))
        ld_x_half(nc.sync, 2, 0)
        ld_x_half(nc.scalar, 1, 1)
        nc.sync.dma_start(out=wv_sb, in_=wv_d[:].rearrange("(o p) m -> p o m", p=P))
        ld_x_2(nc.scalar, 2, 1)
        ld_x_half(nc.sync, 3, 0)
        ld_x_half(nc.sync, 4, 0)
        ld_x_2(nc.scalar, 4, 1)
        ld_x_half(nc.sync, 5, 0)
        ld_x_half(nc.sync, 6, 0)
        ld_x_2(nc.scalar, 6, 1)
        ld_x_half(nc.sync, 7, 0)

        ones_1 = singles.tile([1, P], F32)    # lhsT for K=1 partition broadcast
        nc.vector.memset(ones_1, 1.0)
        ones_b = singles.tile([P, P], BF16)   # all-ones bf16 lhsT: Z colsum
        nc.vector.memset(ones_b, 1.0)

        # broadcast gamma and gamma*vb across partitions via K=1 matmuls
        gamma_bc = singles.tile([P, 1], F32)
        pg = pp.tile([P, 1], F32, tag="pp", name="pg")
        nc.tensor.matmul(pg, ones_1, gamma_sb, start=True, stop=True)
        nc.vector.tensor_copy(gamma_bc, pg)
        gvb_bc = singles.tile([P, C], F32)
        pvb = pp.tile([P, C], F32, tag="pp")
        nc.tensor.matmul(pvb, ones_1, vb_sb, start=True, stop=True)
        nc.vector.tensor_scalar_mul(gvb_bc, pvb, gamma_bc)

        # ---- projections --------------------------------------------------------
        # qq/kk are stored bf16: the QK matmuls then stream 1 cycle/col (vs
        # fp32r's 2) with single-pass weight loads.  Logit error ~1.3% of a
        # unit -- measured end-to-end impact ~1.1e-2 relative (tolerance 2e-2).
        qq_sb = singles.tile([P, NI], BF16)    # [q; q] duplicated across halves
        kk_sb = singles.tile([P, N], BF16)     # [k; k] duplicated across halves
        vt_sb = singles.tile([P, N // P, C], BF16)   # V^T: [j, c], pre-scaled

        def qq_slice(s):
            ps = pp.tile([P, 512], F32, tag="pp", name=f"qq_ps_{s}")
            nc.tensor.matmul(ps, wq_sb[:, 0], x_sb[:, 0, bass.ts(s, 512)],
                             start=True, stop=False)
            nc.tensor.matmul(ps, wq_sb[:, 1], x_sb[:, 1, bass.ts(s, 512)],
                             start=False, stop=True)
            # on Vector, not ScalarE: ScalarE's projection-window budget goes
            # to the kk copies and the Activation-ring DMA issues
            nc.vector.tensor_scalar_add(qq_sb[:, bass.ts(s, 512)], ps, qb_sb)

        def kk_slice(s):
            ps = pp.tile([P, 512], F32, tag="pp", name=f"kk_ps_{s}")
            nc.tensor.matmul(ps, wk_sb[:, 0], x_sb[:, 0, bass.ts(s, 512)],
                             start=True, stop=False)
            nc.tensor.matmul(ps, wk_sb[:, 1], x_sb[:, 1, bass.ts(s, 512)],
                             start=False, stop=True)
            nc.scalar.activation(out=kk_sb[:, bass.ts(s, 512)], in_=ps,
                                 func=IDENT, bias=kb_sb, scale=1.0)

        def vt_chunk(j):
            ps = pp.tile([P, C], F32, tag="pp", name=f"vt_ps_{j}")
            nc.tensor.matmul(ps, x_sb[:, 0, bass.ts(j, P)], wv_sb[:, 0],
                             start=True, stop=False)
            nc.tensor.matmul(ps, x_sb[:, 1, bass.ts(j, P)], wv_sb[:, 1],
                             start=False, stop=True)
            nc.vector.scalar_tensor_tensor(
                out=vt_sb[:, j], in0=ps, scalar=gamma_bc, in1=gvb_bc,
                op0=mybir.AluOpType.mult, op1=mybir.AluOpType.add)

        # queries are columns 0..NI-1 of the rotated x; consume x strictly in
        # chunk-arrival order (kk slice s and vt chunks 4s..4s+3 share chunk s)
        qq_slice(0)
        qq_slice(1)
        for s in range(N // 512):
            kk_slice(s)
            for j in range(4 * s, 4 * s + 4):
                vt_chunk(j)
            if s == 1:
                qq_slice(2)
            elif s == 2:
                qq_slice(3)

        # ---- attention ----------------------------------------------------------
        # Row-packed QK: pair (jA, jB) = (2t, 2t+1); jA on PE rows 0-63, jB on
        # rows 64-127 (via the duplicated q/k halves), running concurrently.
        NPAIR = N // P // 2   # 16 pairs per phase
        NPH = NI // PH        # 2 phases

        def issue_pair(ph, t):
            # One PSUM tile holds both halves' S^T slices ([P, 2, 512]); the
            # two K=64 QK matmuls row-pack (rows 0-63 / 64-127) and a SINGLE
            # [128, 1024] exp covers both halves (amortizes ScalarE's fixed
            # per-instruction overhead -- ScalarE is the near-critical engine).
            i0 = ph * PH
            ps = p_s.tile([P, 2, PH], F32, tag="s", name=f"ps_{ph}_{t}")
            for h, j in ((0, 2 * t), (1, 2 * t + 1)):
                lo = h * CQ
                nc.tensor.matmul(
                    ps[:, h],
                    kk_sb[lo:lo + CQ, bass.ts(j, P)],
                    qq_sb[lo:lo + CQ, bass.ds(i0, PH)],
                    start=True, stop=True)
            et = etp.tile([P, 2, PH], BF16, tag="et", name=f"et_{ph}_{t}")
            nc.scalar.activation(out=et, in_=ps, func=EXP, scale=1.0)
            return et

        def pv_half(po, t, h, et):
            j = 2 * t + h
            for cc in range(C // P):
                nc.tensor.matmul(
                    po[cc],
                    vt_sb[:, j, bass.ts(cc, P)],
                    et[:, h],
                    start=(t == 0 and h == 0), stop=(t == NPAIR - 1 and h == 1))

        def finalize(ph, za, zb, po, et15):
            i0 = ph * PH
            # Z colsum + partition-broadcast via all-ones bf16 lhsT matmuls;
            # the last pair's exp tile is summed directly by the PE (avoids
            # waiting on the accumulation chains).  The reciprocal/scale/add/
            # DMA chain runs on Vector/Pool while the PE begins the next phase
            # (po PSUM banks rotate between phases, so the next phase's PV
            # does not wait on this chain).  The last phase instead keeps the
            # whole chain on Vector (Pool's adds are ~2x slower and would sit
            # on the critical tail) and DMAs each channel half as soon as it
            # is ready.
            last = ph == NPH - 1
            pzb = p_s.tile([P, PH], F32, tag="s", name=f"pzb_{ph}")
            nc.tensor.matmul(pzb, ones_b, za, start=True, stop=False)
            nc.tensor.matmul(pzb, ones_b, zb, start=False, stop=False)
            nc.tensor.matmul(pzb, ones_b, et15[:, 0], start=False, stop=False)
            nc.tensor.matmul(pzb, ones_b, et15[:, 1], start=False, stop=True)
            zbc = zp.tile([P, PH], F32, tag="zbc", name=f"zbc_{ph}")
            nc.vector.reciprocal_approx_fast(out=zbc, in_=pzb)
            sl_i = bass.ds(i0, PH)
            ob = outp.tile([P, 2, PH], F32, tag="ob", name=f"ob_{ph}")
            out_r = out_d[:].rearrange("(o p) n -> p o n", p=P)
            for cc in range(C // P):
                nc.vector.tensor_mul(ob[:, cc], po[cc], zbc)
                if last:
                    nc.vector.tensor_add(ob[:, cc], ob[:, cc],
                                         _f(x_sb[:, cc, sl_i]))
                    nc.sync.dma_start(out=out_r[:, cc, sl_i], in_=ob[:, cc])
                else:
                    nc.gpsimd.tensor_add(ob[:, cc], ob[:, cc],
                                         _f(x_sb[:, cc, sl_i]))
            if not last:
                nc.sync.dma_start(out=out_r[:, :, sl_i], in_=ob)

        # Software pipeline, 1.5 pairs deep: per step, issue QK/exp for pair
        # t+1, then the B-half PV of pair t-1, then the A-half PV of pair t.
        # Deferring each pair's B-half by one step gives exp(t) two extra PV
        # matmuls of slack before PV-A(t) consumes it (exp is ~1.1us in a
        # ~1.35us pair -- without the extra slack the PE idles ~0.4us/pair).
        pend = {(0, 0): issue_pair(0, 0)}
        zacc = {}
        for ph in range(NPH):
            za = zp.tile([P, PH], BF16, tag="za", name=f"za_{ph}")
            zb = zp.tile([P, PH], BF16, tag="zb", name=f"zb_{ph}")
            zacc[ph] = (za, zb)
            po = [pp.tile([P, PH], F32, tag="pp", name=f"po_{ph}_{cc}")
                  for cc in range(C // P)]
            prev = None  # (t, et) whose B-half is still outstanding
            for t in range(NPAIR):
                et = pend.pop((ph, t))
                nxt = (ph, t + 1) if t + 1 < NPAIR else (
                    (ph + 1, 0) if ph + 1 < NPH else None)
                if nxt is not None:
                    pend[nxt] = issue_pair(*nxt)
                if prev is not None:
                    pv_half(po, prev[0], 1, prev[1])
                pv_half(po, t, 0, et)
                prev = (t, et)
                if t == 0:
                    nc.vector.tensor_copy(za, et[:, 0])
                    nc.vector.tensor_copy(zb, et[:, 1])
                elif t < NPAIR - 1:
                    nc.vector.tensor_add(za, za, et[:, 0])
                    nc.vector.tensor_add(zb, zb, et[:, 1])
            pv_half(po, prev[0], 1, prev[1])
            finalize(ph, za, zb, po, prev[1])


def _build_nc():
    nc = bacc.Bacc(trn_type="TRN2", target_bir_lowering=False, debug=False)
    aps = (
        nc.declare_dram_parameter("x", [C, N], MM_DT, isOutput=False),
        nc.declare_dram_parameter("wqT", [C, P], MM_DT, isOutput=False),
        nc.declare_dram_parameter("wkT", [C, P], MM_DT, isOutput=False),
        nc.declare_dram_parameter("wvT", [C, C], MM_DT, isOutput=False),
        nc.declare_dram_parameter("qb", [P, 1], F32, isOutput=False),
        nc.declare_dram_parameter("kb", [P, 1], F32, isOutput=False),
        nc.declare_dram_parameter("vb", [1, C], F32, isOutput=False),
        nc.declare_dram_parameter("gamma", [1, 1], F32, isOutput=False),
        nc.declare_dram_parameter("out", [C, NI], F32, isOutput=True),
    )
    with tile.TileContext(nc) as tc:
        _emit(tc, aps)
    nc.compile()
    return nc


_NC_CACHE = {}


def get_nc():
    if "nc" not in _NC_CACHE:
        _NC_CACHE["nc"] = _build_nc()
    return _NC_CACHE["nc"]


def make_in_maps(inputs):
    """Build the 8 per-core input maps from the full problem inputs."""
    f = np.float32
    x_streams = [
        np.ascontiguousarray(inputs["input1"].reshape(B, C, N), dtype=f),
        np.ascontiguousarray(inputs["input2"].reshape(B, C, N), dtype=f),
    ]
    wsets = []
    for s in ("1", "2"):
        qw = np.asarray(inputs[f"q{s}_w"], dtype=f)
        kw = np.asarray(inputs[f"k{s}_w"], dtype=f)
        vw = np.asarray(inputs[f"v{s}_w"], dtype=f)
        qb = np.asarray(inputs[f"q{s}_b"], dtype=f)
        kb = np.asarray(inputs[f"k{s}_b"], dtype=f)
        vb = np.asarray(inputs[f"v{s}_b"], dtype=f)
        wsets.append(dict(
            wqT=np.ascontiguousarray(np.concatenate([qw, qw], 0).T),
            wkT=np.ascontiguousarray(np.concatenate([kw, kw], 0).T),
            wvT=np.ascontiguousarray(vw.T),
            qb=np.ascontiguousarray(np.concatenate([qb, qb])[:, None]),
            kb=np.ascontiguousarray(np.concatenate([kb, kb])[:, None]),
            vb=np.ascontiguousarray(vb[None, :]),
        ))
    gamma = np.ascontiguousarray(np.asarray(inputs["gamma"], dtype=f).reshape(1, 1))

    in_maps = []
    for core in range(8):
        u, h = core // 2, core % 2
        b, s = u // 2, u % 2
        xs = x_streams[s][b]
        m = dict(wsets[s])
        # rotate so this core's query slice comes first (attention contracts
        # over all keys, so key order is irrelevant)
        if h == 0:
            m["x"] = xs
        else:
            m["x"] = np.ascontiguousarray(
                np.concatenate([xs[:, NI:], xs[:, :NI]], axis=1))
        m["gamma"] = gamma
        in_maps.append(m)
    return in_maps


def assemble(results, inputs):
    """Stitch the 8 per-core [256, 2048] outputs into (out1, out2)."""
    outs = [np.empty((B, C, N), np.float32) for _ in range(2)]
    for core in range(8):
        u, h = core // 2, core % 2
        b, s = u // 2, u % 2
        outs[s][b][:, h * NI:(h + 1) * NI] = results[core]["out"]
    out1 = outs[0].reshape(B, C, H, W)
    out2 = outs[1].reshape(B, C, H, W)
    return out1, out2


def kernel(**inputs):
    from concourse.bass_utils import run_bass_kernel_spmd

    nc = get_nc()
    in_maps = make_in_maps(inputs)
    res = run_bass_kernel_spmd(nc, in_maps, list(range(8)))
    return assemble(res.results, inputs)


# revision 25
# speedup vs baseline: 1.0246x; 1.0246x over previous
"""Trainium2 Bass kernel for nn_AttShare: dual-stream 1x1-conv attention.

Full-input contract: kernel(**inputs) takes the complete tensors from
setup_inputs() and returns (out1, out2) exactly like the reference.

Sharding (8 cores): 4 independent (batch, stream) attention units x 2-way
query-row split.  Each core gets the full x=[256,4096] of its unit, HOST-
ROTATED so its 2048 query columns come first; it produces
out = gamma * (V @ softmax(Q K^T)^T)[:, 0:2048] + x[:, 0:2048].
(Attention contracts over all keys, so the key/value column order is
irrelevant; the host scatters the output back to the right columns.)

Key simplification: the reference adds a per-row bias (q . g) to the logits
before a row-softmax.  softmax is shift-invariant per row, so the entire
global-gating branch (pooled means -> MLP -> sigmoid -> bias) cancels and is
not computed.  The k-projection bias also shifts logits uniformly per row
and cancels; the q bias does not and is applied.  The v bias adds
gamma*vb[c] (softmax rows sum to 1); it is folded into the V^T tiles.

Precision: projections and QK logits run in float32r (full fp32 weights,
~19-bit moving operand) -- logit errors get exponentiated, so this path
stays wide.  The PV path (V^T tiles and exp tiles) runs in bfloat16:
weight loads take 1 pass instead of fp32's 2, cutting the ldweights
exposure between back-to-back PV matmuls.  Measured accuracy impact
~1.4e-3 relative (tolerance 2e-2).

On-core dataflow (per core):
  proj:  qq = Wq_dup @ x[:, :2048] (+qb)  [128, 2048] f32r  (q/k duplicated
         kk = Wk_dup @ x  (+kb)           [128, 4096] f32r   on both halves
         vt = gamma*(x^T @ Wv^T + vb)     [128 j, 32, 256] bf16  for packing)
  attn (2 phases of 1024 query columns, j streamed in row-packed pairs,
        software-pipelined one pair ahead):
         S^T tile = kk_j^T @ qq  (K=64, rows 0-63 / 64-127 concurrently)
         E = exp(S^T)  (ScalarE, PSUM -> bf16 SBUF; no max-shift needed:
                        |S|<~60 and the denominator normalizes later)
         ZA += E_A (Pool)   ZB += E_B (Vector)   [split across engines]
         out_psum[c,i] += vt_j^T @ E  (bf16 matmuls, PSUM-resident)
  finalize per 512-col slice (pipelined against the next phase / the last
  PV matmuls): Z colsum+broadcast via all-ones lhsT matmuls (4 terms: ZA,
  ZB and the last pair's exp tiles summed directly by the PE), reciprocal
  (Vector), out = out_psum * recip (Vector) + x (Pool), DMA out (SP/ACT
  queues alternating).
  PSUM budget 8 banks: 4 output accumulators + 2x2-bank S^T tiles.

Head: input DMA issues are split across the SP and Activation hardware
queues (plus Pool's software queue for the scalars) so descriptor
generation (~0.7us each) does not serialize the x stream.
"""

import os
import sys

import numpy as np

for _p in ("/opt/trn_rl_repo", os.path.expanduser("~/.axon_site/_ro/trn_rl_repo")):
    if os.path.isdir(_p) and _p not in sys.path:
        sys.path.insert(0, _p)

import concourse.bass as bass  # noqa: E402
import concourse.bacc as bacc  # noqa: E402
import concourse.mybir as mybir  # noqa: E402
import concourse.tile as tile  # noqa: E402

P = 128
C = 256         # channels
CQ = 64         # q/k channels
N = 4096        # H*W
NI = 2048       # query rows per core
PH = 512        # query columns processed per phase
B, H, W = 2, 64, 64
F32 = mybir.dt.float32
BF16 = mybir.dt.bfloat16
MM_DT = mybir.dt.float32r


def _f(ap):
    """View a float32r AP as plain fp32 (for non-matmul engine access)."""
    return ap.bitcast(F32)


def _r(ap):
    """View an fp32 AP as float32r (for matmul operands)."""
    return ap.bitcast(MM_DT)


def _emit(tc, aps):
    nc = tc.nc
    import contextlib

    x_d, wq_d, wk_d, wv_d, qb_d, kb_d, vb_d, gamma_d, out_d = aps
    EXP = mybir.ActivationFunctionType.Exp
    IDENT = mybir.ActivationFunctionType.Identity

    with contextlib.ExitStack() as ctx:
        singles = ctx.enter_context(tc.tile_pool(name="singles", bufs=1))
        pp = ctx.enter_context(tc.tile_pool(name="pp", bufs=4, space="PSUM"))
        p_s = ctx.enter_context(tc.tile_pool(name="p_s", bufs=2, space="PSUM"))
        etp = ctx.enter_context(tc.tile_pool(name="etp", bufs=8))
        zp = ctx.enter_context(tc.tile_pool(name="zp", bufs=4))
        outp = ctx.enter_context(tc.tile_pool(name="outp", bufs=4))

        # ---- loads --------------------------------------------------------------
        x_sb = singles.tile([P, 2, N], MM_DT)
        wq_sb = singles.tile([P, 2, P], MM_DT)
        wk_sb = singles.tile([P, 2, P], MM_DT)
        wv_sb = singles.tile([P, 2, C], MM_DT)
        x_r = x_d[:].rearrange("(o p) n -> p o n", p=P)

        gamma_sb = singles.tile([1, 1], F32)
        kb_sb = singles.tile([P, 1], F32)
        qb_sb = singles.tile([P, 1], F32)
        vb_sb = singles.tile([1, C], F32)

        # gamma/vb lead the SP queue (tiny transfers, unblock the HAM-warming
        # broadcast matmuls); qb/kb ride the Pool software queue.  The x
        # stream is split by cin-half across the two hardware DMA rings
        # (SP and Activation, ~150GB/s each) so each chunk's halves arrive
        # together at the aggregate ~300GB/s.
        def ld_x_half(queue, c, o):
            queue.dma_start(out=x_sb[:, o:o + 1, bass.ts(c, N // 8)],
                            in_=x_r[:, o:o + 1, bass.ts(c, N // 8)])

        ld_x_half(nc.sync, 0, 0)
        nc.sync.dma_start(out=gamma_sb, in_=gamma_d[:])
        nc.sync.dma_start(out=vb_sb, in_=vb_d[:])
        nc.gpsimd.dma_start(out=qb_sb, in_=qb_d[:])
        nc.gpsimd.dma_start(out=kb_sb, in_=kb_d[:])
        nc.scalar.dma_start(out=wq_sb, in_=wq_d[:].rearrange("(o p) m -> p o m", p=P))
        ld_x_half(nc.scalar, 0, 1)
        ld_x_half(nc.sync, 1, 0)
        nc.scalar.dma_start(out=wk_sb, in_=wk_d[:].rearrange("(o p) m -> p o m", p=P))
        ld_x_half(nc.sync, 2, 0)
        ld_x_half(nc.scalar, 1, 1)
        nc.sync.dma_start(out=wv_sb, in_=wv_d[:].rearrange("(o p) m -> p o m", p=P))
        ld_x_half(nc.scalar, 2, 1)
        for c in range(3, 8):
            ld_x_half(nc.sync, c, 0)
            ld_x_half(nc.scalar, c, 1)

        ones_1 = singles.tile([1, P], F32)    # lhsT for K=1 partition broadcast
        nc.vector.memset(ones_1, 1.0)
        ones_b = singles.tile([P, P], BF16)   # all-ones bf16 lhsT: Z colsum
        nc.vector.memset(ones_b, 1.0)

        # broadcast gamma and gamma*vb across partitions via K=1 matmuls
        gamma_bc = singles.tile([P, 1], F32)
        pg = pp.tile([P, 1], F32, tag="pp", name="pg")
        nc.tensor.matmul(pg, ones_1, gamma_sb, start=True, stop=True)
        nc.vector.tensor_copy(gamma_bc, pg)
        gvb_bc = singles.tile([P, C], F32)
        pvb = pp.tile([P, C], F32, tag="pp")
        nc.tensor.matmul(pvb, ones_1, vb_sb, start=True, stop=True)
        nc.vector.tensor_scalar_mul(gvb_bc, pvb, gamma_bc)

        # ---- projections --------------------------------------------------------
        # qq/kk are stored bf16: the QK matmuls then stream 1 cycle/col (vs
        # fp32r's 2) with single-pass weight loads.  Logit error ~1.3% of a
        # unit -- measured end-to-end impact ~1.1e-2 relative (tolerance 2e-2).
        qq_sb = singles.tile([P, NI], BF16)    # [q; q] duplicated across halves
        kk_sb = singles.tile([P, N], BF16)     # [k; k] duplicated across halves
        vt_sb = singles.tile([P, N // P, C], BF16)   # V^T: [j, c], pre-scaled

        def qq_slice(s):
            ps = pp.tile([P, 512], F32, tag="pp", name=f"qq_ps_{s}")
            nc.tensor.matmul(ps, wq_sb[:, 0], x_sb[:, 0, bass.ts(s, 512)],
                             start=True, stop=False)
            nc.tensor.matmul(ps, wq_sb[:, 1], x_sb[:, 1, bass.ts(s, 512)],
                             start=False, stop=True)
            nc.scalar.activation(out=qq_sb[:, bass.ts(s, 512)], in_=ps,
                                 func=IDENT, bias=qb_sb, scale=1.0)

        def kk_slice(s):
            ps = pp.tile([P, 512], F32, tag="pp", name=f"kk_ps_{s}")
            nc.tensor.matmul(ps, wk_sb[:, 0], x_sb[:, 0, bass.ts(s, 512)],
                             start=True, stop=False)
            nc.tensor.matmul(ps, wk_sb[:, 1], x_sb[:, 1, bass.ts(s, 512)],
                             start=False, stop=True)
            nc.scalar.activation(out=kk_sb[:, bass.ts(s, 512)], in_=ps,
                                 func=IDENT, bias=kb_sb, scale=1.0)

        def vt_chunk(j):
            ps = pp.tile([P, C], F32, tag="pp", name=f"vt_ps_{j}")
            nc.tensor.matmul(ps, x_sb[:, 0, bass.ts(j, P)], wv_sb[:, 0],
                             start=True, stop=False)
            nc.tensor.matmul(ps, x_sb[:, 1, bass.ts(j, P)], wv_sb[:, 1],
                             start=False, stop=True)
            nc.vector.scalar_tensor_tensor(
                out=vt_sb[:, j], in0=ps, scalar=gamma_bc, in1=gvb_bc,
                op0=mybir.AluOpType.mult, op1=mybir.AluOpType.add)

        # queries are columns 0..NI-1 of the rotated x; consume x strictly in
        # chunk-arrival order (kk slice s and vt chunks 4s..4s+3 share chunk s)
        qq_slice(0)
        qq_slice(1)
        for s in range(N // 512):
            kk_slice(s)
            for j in range(4 * s, 4 * s + 4):
                vt_chunk(j)
            if s == 1:
                qq_slice(2)
            elif s == 2:
                qq_slice(3)

        # ---- attention ----------------------------------------------------------
        # Row-packed QK: pair (jA, jB) = (2t, 2t+1); jA on PE rows 0-63, jB on
        # rows 64-127 (via the duplicated q/k halves), running concurrently.
        NPAIR = N // P // 2   # 16 pairs per phase
        NPH = NI // PH        # 2 phases

        def issue_pair(ph, t):
            # One PSUM tile holds both halves' S^T slices ([P, 2, 512]); the
            # two K=64 QK matmuls row-pack (rows 0-63 / 64-127) and a SINGLE
            # [128, 1024] exp covers both halves (amortizes ScalarE's fixed
            # per-instruction overhead -- ScalarE is the near-critical engine).
            i0 = ph * PH
            ps = p_s.tile([P, 2, PH], F32, tag="s", name=f"ps_{ph}_{t}")
            for h, j in ((0, 2 * t), (1, 2 * t + 1)):
                lo = h * CQ
                nc.tensor.matmul(
                    ps[:, h],
                    kk_sb[lo:lo + CQ, bass.ts(j, P)],
                    qq_sb[lo:lo + CQ, bass.ds(i0, PH)],
                    start=True, stop=True)
            et = etp.tile([P, 2, PH], BF16, tag="et", name=f"et_{ph}_{t}")
            nc.scalar.activation(out=et, in_=ps, func=EXP, scale=1.0)
            return et

        def pv_half(po, t, h, et):
            j = 2 * t + h
            for cc in range(C // P):
                nc.tensor.matmul(
                    po[cc],
                    vt_sb[:, j, bass.ts(cc, P)],
                    et[:, h],
                    start=(t == 0 and h == 0), stop=(t == NPAIR - 1 and h == 1))

        def finalize_z(ph, za, zb, et15):
            # Z colsum + partition-broadcast via all-ones bf16 lhsT matmuls;
            # the last pair's exp tile is summed directly by the PE (avoids
            # waiting on the accumulation chains).  Emitted between the last
            # pair's two PV halves so the reciprocal overlaps the remaining
            # PV stream.
            pzb = p_s.tile([P, PH], F32, tag="s", name=f"pzb_{ph}")
            nc.tensor.matmul(pzb, ones_b, za, start=True, stop=False)
            nc.tensor.matmul(pzb, ones_b, zb, start=False, stop=False)
            nc.tensor.matmul(pzb, ones_b, et15[:, 0], start=False, stop=False)
            nc.tensor.matmul(pzb, ones_b, et15[:, 1], start=False, stop=True)
            zbc = zp.tile([P, PH], F32, tag="zbc", name=f"zbc_{ph}")
            nc.vector.reciprocal_approx_fast(out=zbc, in_=pzb)
            return zbc

        def finalize_out(ph, po, zbc):
            # The scale/add/DMA chain runs on Vector/Pool while the PE begins
            # the next phase (po PSUM banks rotate between phases, so the next
            # phase's PV does not wait on this chain).  The last phase keeps
            # the adds on Vector (Pool's adds are ~2x slower and would sit on
            # the critical tail) and DMAs each channel half as soon as ready.
            last = ph == NPH - 1
            sl_i = bass.ds(ph * PH, PH)
            ob = outp.tile([P, 2, PH], F32, tag="ob", name=f"ob_{ph}")
            out_r = out_d[:].rearrange("(o p) n -> p o n", p=P)
            for cc in range(C // P):
                nc.vector.tensor_mul(ob[:, cc], po[cc], zbc)
                if last:
                    nc.vector.tensor_add(ob[:, cc], ob[:, cc],
                                         _f(x_sb[:, cc, sl_i]))
                    nc.sync.dma_start(out=out_r[:, cc, sl_i], in_=ob[:, cc])
                else:
                    nc.gpsimd.tensor_add(ob[:, cc], ob[:, cc],
                                         _f(x_sb[:, cc, sl_i]))
            if not last:
                nc.sync.dma_start(out=out_r[:, :, sl_i], in_=ob)

        # Software pipeline, 1.5 pairs deep: per step, issue QK/exp for pair
        # t+1, then the B-half PV of pair t-1, then the A-half PV of pair t.
        # Deferring each pair's B-half by one step gives exp(t) two extra PV
        # matmuls of slack before PV-A(t) consumes it (exp is ~1.1us in a
        # ~1.35us pair -- without the extra slack the PE idles ~0.4us/pair).
        pend = {(0, 0): issue_pair(0, 0)}
        zacc = {}
        for ph in range(NPH):
            za = zp.tile([P, PH], BF16, tag="za", name=f"za_{ph}")
            zb = zp.tile([P, PH], BF16, tag="zb", name=f"zb_{ph}")
            zacc[ph] = (za, zb)
            po = [pp.tile([P, PH], F32, tag="pp", name=f"po_{ph}_{cc}")
                  for cc in range(C // P)]
            prev = None  # (t, et) whose B-half is still outstanding
            for t in range(NPAIR):
                et = pend.pop((ph, t))
                nxt = (ph, t + 1) if t + 1 < NPAIR else (
                    (ph + 1, 0) if ph + 1 < NPH else None)
                if nxt is not None:
                    pend[nxt] = issue_pair(*nxt)
                if prev is not None:
                    pv_half(po, prev[0], 1, prev[1])
                pv_half(po, t, 0, et)
                prev = (t, et)
                if t == 0:
                    nc.vector.tensor_copy(za, et[:, 0])
                    nc.vector.tensor_copy(zb, et[:, 1])
                elif t < NPAIR - 1:
                    nc.vector.tensor_add(za, za, et[:, 0])
                    nc.vector.tensor_add(zb, zb, et[:, 1])
            zbc = finalize_z(ph, za, zb, prev[1])
            pv_half(po, prev[0], 1, prev[1])
            finalize_out(ph, po, zbc)


def _build_nc():
    nc = bacc.Bacc(trn_type="TRN2", target_bir_lowering=False, debug=False)
    aps = (
        nc.declare_dram_parameter("x", [C, N], MM_DT, isOutput=False),
        nc.declare_dram_parameter("wqT", [C, P], MM_DT, isOutput=False),
        nc.declare_dram_parameter("wkT", [C, P], MM_DT, isOutput=False),
        nc.declare_dram_parameter("wvT", [C, C], MM_DT, isOutput=False),
        nc.declare_dram_parameter("qb", [P, 1], F32, isOutput=False),
        nc.declare_dram_parameter("kb", [P, 1], F32, isOutput=False),
        nc.declare_dram_parameter("vb", [1, C], F32, isOutput=False),
        nc.declare_dram_parameter("gamma", [1, 1], F32, isOutput=False),
        nc.declare_dram_parameter("out", [C, NI], F32, isOutput=True),
    )
    with tile.TileContext(nc) as tc:
        _emit(tc, aps)
    nc.compile()
    return nc


_NC_CACHE = {}


def get_nc():
    if "nc" not in _NC_CACHE:
        _NC_CACHE["nc"] = _build_nc()
    return _NC_CACHE["nc"]


def make_in_maps(inputs):
    """Build the 8 per-core input maps from the full problem inputs."""
    f = np.float32
    x_streams = [
        np.ascontiguousarray(inputs["input1"].reshape(B, C, N), dtype=f),
        np.ascontiguousarray(inputs["input2"].reshape(B, C, N), dtype=f),
    ]
    wsets = []
    for s in ("1", "2"):
        qw = np.asarray(inputs[f"q{s}_w"], dtype=f)
        kw = np.asarray(inputs[f"k{s}_w"], dtype=f)
        vw = np.asarray(inputs[f"v{s}_w"], dtype=f)
        qb = np.asarray(inputs[f"q{s}_b"], dtype=f)
        kb = np.asarray(inputs[f"k{s}_b"], dtype=f)
        vb = np.asarray(inputs[f"v{s}_b"], dtype=f)
        wsets.append(dict(
            wqT=np.ascontiguousarray(np.concatenate([qw, qw], 0).T),
            wkT=np.ascontiguousarray(np.concatenate([kw, kw], 0).T),
            wvT=np.ascontiguousarray(vw.T),
            qb=np.ascontiguousarray(np.concatenate([qb, qb])[:, None]),
            kb=np.ascontiguousarray(np.concatenate([kb, kb])[:, None]),
            vb=np.ascontiguousarray(vb[None, :]),
        ))
    gamma = np.ascontiguousarray(np.asarray(inputs["gamma"], dtype=f).reshape(1, 1))

    in_maps = []
    for core in range(8):
        u, h = core // 2, core % 2
        b, s = u // 2, u % 2
        xs = x_streams[s][b]
        m = dict(wsets[s])
        # rotate so this core's query slice comes first (attention contracts
        # over all keys, so key order is irrelevant)
        if h == 0:
            m["x"] = xs
        else:
            m["x"] = np.ascontiguousarray(
                np.concatenate([xs[:, NI:], xs[:, :NI]], axis=1))
        m["gamma"] = gamma
        in_maps.append(m)
    return in_maps


def assemble(results, inputs):
    """Stitch the 8 per-core [256, 2048] outputs into (out1, out2)."""
    outs = [np.empty((B, C, N), np.float32) for _ in range(2)]
    for core in range(8):
        u, h = core // 2, core % 2
        b, s = u // 2, u % 2
        outs[s][b][:, h * NI:(h + 1) * NI] = results[core]["out"]
    out1 = outs[0].reshape(B, C, H, W)
    out2 = outs[1].reshape(B, C, H, W)
    return out1, out2


def kernel(**inputs):
    from concourse.bass_utils import run_bass_kernel_spmd

    nc = get_nc()
    in_maps = make_in_maps(inputs)
    res = run_bass_kernel_spmd(nc, in_maps, list(range(8)))
    return assemble(res.results, inputs)
